# revision 7
# baseline (speedup 1.0000x reference)
"""Trainium2 Bass kernel for the neural-ODE Euler integration problem.

Key idea: collapse the per-step x -> L1 -> ... -> L4 -> Euler -> x loop into a
PSUM-resident recurrence on the layer-1 preactivation a1 [256, PB]:

    a1_{i+1} = a1_i + s3_i @ G + Uz . dz_i + c_G        (G = dt * W4 @ Ux)

where s3 = elu(a3)+1 is the (shifted) last hidden activation.  The x
trajectory accumulates separately in PSUM (x_{i+1} = x_i + s3_i @ (dt*W4)),
off the critical path; the constant bias drift (t_i - t_0) * b4 is added
on the host during unpack.

elu(v)+1 = relu(v) + 1 + min(exp(v)-1, 0)   (exact), fed to the next layer
as TWO matmul rhs streams so no combine op sits on the critical path:
    ScalarE:  e  = exp(v + b)            (PSUM -> SBUF fp16)
    VectorE:  r  = relu(v + b)           (PSUM -> SBUF fp16)
    GpSimd:   n' = min(e - 1, 0)         (SBUF -> SBUF fp16, off the bank)
    next layer: s@W = r@W + n'@W + colsum(W)  (colsum folded into biases)
Each PSUM tile owns a full bank to avoid collision stalls.
"""

import numpy as np
import sys

if '/opt/trn_rl_repo' not in sys.path:
    sys.path.insert(0, '/opt/trn_rl_repo')

import concourse.bass as bass
import concourse.bacc as bacc
import concourse.mybir as mybir
from concourse.tile import TileContext
from concourse import bass_utils

F32 = mybir.dt.float32
F16 = mybir.dt.float16
AF = mybir.ActivationFunctionType
OP = mybir.AluOpType

B, T, XD, ZD, HID = 1024, 1000, 8, 8, 256
NCORES = 8
PB = B // NCORES          # 128 trajectories per core
SPG = 16                  # steps per output ring group
NG = 63                   # 63*16 = 1008 slots >= 999
NSTEPS = T - 1

LAST_RESULTS = None


def _build(nsteps, ng):
    nc = bacc.Bacc("TRN2", target_bir_lowering=False, debug=False,
                   num_devices=NCORES)
    ncols = ng * PB
    nslot = ng * SPG

    d = {}
    def din(name, shape, dt):
        d[name] = nc.dram_tensor(name, shape, dt, kind="ExternalInput").ap()
    din("dzpack", [128, ncols], F16)    # packed z_eff diffs
    din("uzd",   [128, 16 * 256], F16)  # 16 slot variants of Uz rows
    din("gw",    [128, 512], F16)       # G = dt*W4@Ux, chunk (kc,h)
    din("w2p",   [128, 512], F16)
    din("w3p",   [128, 512], F16)
    din("w4d",   [128, 16], F16)        # dt*W4, chunk kc
    din("cgl",   [128, 256], F16)       # c_G hi/lo rows, per half
    din("a10hi", [128, 2 * PB], F16)    # a1_0 halves
    din("a10lo", [128, 2 * PB], F16)
    din("x0hi",  [8, PB], F16)
    din("x0lo",  [8, PB], F16)
    din("ones2", [128, PB], F16)        # rows 0-1 = 1
    din("ident", [128, 128], F16)
    din("bvec",  [128, 8], F32)         # bias columns (see host)
    xout_d = nc.dram_tensor("xout_d", [8, nslot + SPG, PB], F32,
                            kind="ExternalOutput").ap()

    with TileContext(nc) as tc:
        with tc.tile_pool(name="const", bufs=1) as cpool, \
             tc.tile_pool(name="work", bufs=4) as wpool, \
             tc.tile_pool(name="psum", bufs=1, space="PSUM") as ppool:

            sb = {}
            for name in d:
                shape = [int(s) for s in d[name].shape]
                sb[name] = cpool.tile(shape, d[name].dtype, name=name, tag=name)
                nc.sync.dma_start(out=sb[name][:], in_=d[name])

            # PSUM: one full bank per tile
            pa = {}
            for nm in ("a1h0", "a1h1", "a2h0", "a2h1", "a3h0", "a3h1", "x"):
                pa[nm] = ppool.tile([128, 512], F32, name=nm, tag=nm)

            a1 = [pa["a1h0"][:, 0:PB], pa["a1h1"][:, 0:PB]]
            a2 = [pa["a2h0"][:, 0:PB], pa["a2h1"][:, 0:PB]]
            a3 = [pa["a3h0"][:, 0:PB], pa["a3h1"][:, 0:PB]]
            px = pa["x"][0:8, 0:PB]

            bexp = {2: [sb["bvec"][:, 0:1], sb["bvec"][:, 1:2]],
                    3: [sb["bvec"][:, 4:5], sb["bvec"][:, 5:6]]}
            brel = {2: [sb["bvec"][:, 2:3], sb["bvec"][:, 3:4]],
                    3: [sb["bvec"][:, 6:7], sb["bvec"][:, 7:8]]}

            # ---- init: a1_0 and x_0 into PSUM via identity matmuls ----
            for h in range(2):
                hs = slice(h * PB, (h + 1) * PB)
                nc.tensor.matmul(a1[h], lhsT=sb["ident"][:],
                                 rhs=sb["a10hi"][:, hs], start=True, stop=False)
                nc.tensor.matmul(a1[h], lhsT=sb["ident"][:],
                                 rhs=sb["a10lo"][:, hs], start=False, stop=True)
            nc.tensor.matmul(px, lhsT=sb["ident"][0:8, 0:8],
                             rhs=sb["x0hi"][:], start=True, stop=False)
            nc.tensor.matmul(px, lhsT=sb["ident"][0:8, 0:8],
                             rhs=sb["x0lo"][:], start=False, stop=True)

            rings = {}

            def acts(layer, src, tag):
                """emit e (ACT), r = relu(v+b) (DVE), n' = min(e-1,0) (POOL);
                return (r, n) stream tiles."""
                e, r, n = [], [], []
                for h in range(2):
                    e.append(wpool.tile([128, PB], F16, name=f"e{tag}{h}",
                                        tag=f"e{layer}h{h}"))
                    r.append(wpool.tile([128, PB], F16, name=f"r{tag}{h}",
                                        tag=f"r{layer}h{h}"))
                    n.append(wpool.tile([128, PB], F16, name=f"n{tag}{h}",
                                        tag=f"n{layer}h{h}"))
                for h in range(2):
                    if layer == 1:
                        nc.scalar.activation(e[h][:], src[h], AF.Exp)
                    else:
                        nc.scalar.activation(e[h][:], src[h], AF.Exp,
                                             bias=bexp[layer][h])
                for h in range(2):
                    if layer == 1:
                        nc.vector.tensor_scalar_max(r[h][:], src[h], 0.0)
                    else:
                        nc.vector.tensor_scalar(
                            out=r[h][:], in0=src[h], scalar1=bexp[layer][h],
                            scalar2=0.0, op0=OP.add, op1=OP.max)
                for h in range(2):
                    nc.gpsimd.tensor_scalar(
                        out=n[h][:], in0=e[h][:], scalar1=-1.0,
                        scalar2=0.0, op0=OP.add, op1=OP.min)
                return r, n

            for i in range(nsteps):
                g, st = i // SPG, i % SPG
                gc = slice(g * PB, (g + 1) * PB)

                # ---- round 1: activations of a1 ----
                r1, n1 = acts(1, a1, f"1_{i}")

                # ---- layer-2 matmuls + a1 constant updates ----
                # kc0 group (gated on h0-half streams), r before n'
                for h in range(2):
                    w = sb["w2p"][:, h * 128:(h + 1) * 128]
                    nc.tensor.matmul(a2[h], lhsT=w, rhs=r1[0][:],
                                     start=True, stop=False)
                    nc.tensor.matmul(a2[h], lhsT=w, rhs=n1[0][:],
                                     start=False, stop=False)
                nc.tensor.matmul(a1[0], lhsT=sb["cgl"][:, 0:128],
                                 rhs=sb["ones2"][:], start=False, stop=False,
                                 skip_group_check=True)
                nc.tensor.matmul(
                    a1[0],
                    lhsT=sb["uzd"][:, st * 256:st * 256 + 128],
                    rhs=sb["dzpack"][:, gc], start=False, stop=False,
                    skip_group_check=True)
                # kc1 group: r's gate lands last -> emit r first per half
                for h in range(2):
                    w = sb["w2p"][:, (2 + h) * 128:(3 + h) * 128]
                    nc.tensor.matmul(a2[h], lhsT=w, rhs=r1[1][:],
                                     start=False, stop=False)
                    nc.tensor.matmul(a2[h], lhsT=w, rhs=n1[1][:],
                                     start=False, stop=True)
                nc.tensor.matmul(a1[1], lhsT=sb["cgl"][:, 128:256],
                                 rhs=sb["ones2"][:], start=False, stop=False,
                                 skip_group_check=True)
                nc.tensor.matmul(
                    a1[1],
                    lhsT=sb["uzd"][:, st * 256 + 128:st * 256 + 256],
                    rhs=sb["dzpack"][:, gc], start=False, stop=False,
                    skip_group_check=True)

                # ---- round 2 ----
                r2, n2 = acts(2, a2, f"2_{i}")
                for h in range(2):
                    w = sb["w3p"][:, h * 128:(h + 1) * 128]
                    nc.tensor.matmul(a3[h], lhsT=w, rhs=r2[0][:],
                                     start=True, stop=False)
                    nc.tensor.matmul(a3[h], lhsT=w, rhs=n2[0][:],
                                     start=False, stop=False)
                for h in range(2):
                    w = sb["w3p"][:, (2 + h) * 128:(3 + h) * 128]
                    nc.tensor.matmul(a3[h], lhsT=w, rhs=r2[1][:],
                                     start=False, stop=False)
                    nc.tensor.matmul(a3[h], lhsT=w, rhs=n2[1][:],
                                     start=False, stop=True)

                # ---- round 3 ----
                r3, n3 = acts(3, a3, f"3_{i}")

                # ---- a1 G-update + x update ----
                for h in range(2):
                    w = sb["gw"][:, h * 128:(h + 1) * 128]
                    nc.tensor.matmul(a1[h], lhsT=w, rhs=r3[0][:],
                                     start=False, stop=False,
                                     skip_group_check=True)
                    nc.tensor.matmul(a1[h], lhsT=w, rhs=n3[0][:],
                                     start=False, stop=False,
                                     skip_group_check=True)
                nc.tensor.matmul(px, lhsT=sb["w4d"][:, 0:8],
                                 rhs=r3[0][:], start=False, stop=False,
                                 skip_group_check=True)
                nc.tensor.matmul(px, lhsT=sb["w4d"][:, 0:8],
                                 rhs=n3[0][:], start=False, stop=False,
                                 skip_group_check=True)
                for h in range(2):
                    w = sb["gw"][:, (2 + h) * 128:(3 + h) * 128]
                    nc.tensor.matmul(a1[h], lhsT=w, rhs=r3[1][:],
                                     start=False, stop=False,
                                     skip_group_check=True)
                    nc.tensor.matmul(a1[h], lhsT=w, rhs=n3[1][:],
                                     start=False, stop=True,
                                     skip_group_check=True)
                nc.tensor.matmul(px, lhsT=sb["w4d"][:, 8:16],
                                 rhs=n3[1][:], start=False, stop=False,
                                 skip_group_check=True)
                nc.tensor.matmul(px, lhsT=sb["w4d"][:, 8:16],
                                 rhs=r3[1][:], start=False, stop=True,
                                 skip_group_check=True)

                # ---- x_{i+1} into output ring ----
                if g not in rings:
                    rings[g] = wpool.tile([8, SPG * PB], F32, name="xring",
                                          tag="xring", bufs=2)
                nc.vector.tensor_copy(out=rings[g][:, st * PB:(st + 1) * PB],
                                      in_=px)
                if st == SPG - 1 or i == nsteps - 1:
                    nfill = st + 1
                    nc.sync.dma_start(
                        out=xout_d[:, g * SPG + 1:g * SPG + 1 + nfill, :],
                        in_=rings[g][:, :nfill * PB])

    nc.compile()
    return nc


_BUILD_CACHE = {}


def _get_compiled(nsteps, ng):
    key = (nsteps, ng)
    if key not in _BUILD_CACHE:
        _BUILD_CACHE[key] = _build(nsteps, ng)
    return _BUILD_CACHE[key]


def _pack_stream(a, ng):
    """[PB, ng*16, 8] -> [128, ng*128]: row s*8+f, col g*128+j."""
    pb = a.shape[0]
    return (a.transpose(1, 2, 0)
             .reshape(ng, SPG, 8, pb)
             .transpose(1, 2, 0, 3)
             .reshape(128, ng * pb))


def _hilo16(v):
    hi = v.astype(np.float16)
    lo = (v.astype(np.float32) - hi.astype(np.float32)).astype(np.float16)
    return hi, lo


def make_in_maps(t, x, z, event_t, z_jump, W1, b1, W2, b2, W3, b3, W4, b4,
                 nsteps=NSTEPS, ng=NG):
    t = np.asarray(t, np.float32); x = np.asarray(x, np.float32)
    z = np.asarray(z, np.float32)
    event_t = np.asarray(event_t, np.float32)
    z_jump = np.asarray(z_jump, np.float32)
    W1 = np.asarray(W1, np.float32); b1 = np.asarray(b1, np.float32)
    W2 = np.asarray(W2, np.float32); b2 = np.asarray(b2, np.float32)
    W3 = np.asarray(W3, np.float32); b3 = np.asarray(b3, np.float32)
    W4 = np.asarray(W4, np.float32); b4 = np.asarray(b4, np.float32)

    nslot = ng * SPG
    tv = t[0, :, 0]
    dtc = np.float32((tv[nsteps] - tv[0]) / nsteps)

    W1a, W1b, W1c = W1[0:16], W1[16:32], W1[32:48]
    V = (W1a - W1b).astype(np.float64)
    U = (W1b + W1c).astype(np.float64)
    Ux, Uz = U[:8], U[8:16]

    G = dtc * (W4.astype(np.float64) @ Ux)                    # [256, 256]
    cG = dtc * (b4.astype(np.float64) @ Ux)                   # [256]
    cGhi, cGlo = _hilo16(cG)

    gw = G.reshape(2, 128, 2, 128).transpose(1, 0, 2, 3).reshape(128, 512)
    w2p = W2.reshape(2, 128, 2, 128).transpose(1, 0, 2, 3).reshape(128, 512)
    w3p = W3.reshape(2, 128, 2, 128).transpose(1, 0, 2, 3).reshape(128, 512)
    w4d = (dtc * W4).reshape(2, 128, XD).transpose(1, 0, 2).reshape(128, 16)

    cgl = np.zeros((128, 256), np.float16)
    for h in range(2):
        cgl[0, h * 128:(h + 1) * 128] = cGhi[h * 128:(h + 1) * 128]
        cgl[1, h * 128:(h + 1) * 128] = cGlo[h * 128:(h + 1) * 128]

    uz16 = Uz.astype(np.float16)
    uzd = np.zeros((128, 16, 256), np.float16)
    for r in range(16):
        uzd[8 * r:8 * r + 8, r] = uz16
    uzd = uzd.reshape(128, 16 * 256)

    # s = r + n' + 1: the ones-stream colsum cancels the elu-shift colsum,
    # so layer biases stay plain b2/b3.
    b2e = b2.astype(np.float32)
    b3e = b3.astype(np.float32)
    bvec = np.zeros((128, 8), np.float32)
    bvec[:, 0] = b2e[0:128];  bvec[:, 1] = b2e[128:256]
    bvec[:, 4] = b3e[0:128];  bvec[:, 5] = b3e[128:256]

    ident = np.eye(128, dtype=np.float16)
    ones2 = np.concatenate([np.ones((2, PB), np.float16),
                            np.zeros((126, PB), np.float16)], axis=0)

    shared = dict(gw=gw.astype(np.float16), w2p=w2p.astype(np.float16),
                  w3p=w3p.astype(np.float16), w4d=w4d.astype(np.float16),
                  cgl=cgl, uzd=uzd, bvec=bvec, ident=ident, ones2=ones2)

    # z_eff diffs per trajectory
    nz = min(nsteps, T - 1)
    zeff = np.where(tv[None, :nz, None] >= event_t[:, :, None],
                    z_jump[:, None, :], z[:, :nz])
    dz_full = np.zeros((B, nslot, ZD), np.float32)
    dz_full[:, :nz - 1] = zeff[:, 1:] - zeff[:, :-1]

    in_maps = []
    for c in range(NCORES):
        bs = slice(c * PB, (c + 1) * PB)
        x0 = x[bs, 0]
        z0 = z[bs, 0]
        all_init = np.concatenate([x0, z0], -1).astype(np.float64)
        a10 = (all_init @ (V + U) + b1).astype(np.float32)    # [PB, 256]
        a10hi, a10lo = _hilo16(a10.T)                          # [256, PB]
        a10hi2 = np.concatenate([a10hi[0:128], a10hi[128:256]], axis=1)
        a10lo2 = np.concatenate([a10lo[0:128], a10lo[128:256]], axis=1)
        x0hi, x0lo = _hilo16(x0.T)
        m = dict(shared)
        m.update(dzpack=_pack_stream(dz_full[bs].astype(np.float16), ng),
                 a10hi=a10hi2, a10lo=a10lo2, x0hi=x0hi, x0lo=x0lo)
        in_maps.append({k: np.ascontiguousarray(v) for k, v in m.items()})
    return in_maps, tv, b4.astype(np.float64)


def kernel(t, x, z, event_t, z_jump, W1, b1, W2, b2, W3, b3, W4, b4,
           nsteps=NSTEPS, ng=NG):
    global LAST_RESULTS
    x = np.asarray(x, np.float32)
    in_maps, tv, b4eff = make_in_maps(
        t, x, z, event_t, z_jump, W1, b1, W2, b2, W3, b3, W4, b4,
        nsteps=nsteps, ng=ng)

    nc = _get_compiled(nsteps, ng)
    res = bass_utils.run_bass_kernel_spmd(nc, in_maps,
                                          core_ids=list(range(NCORES)))
    LAST_RESULTS = res

    out = np.zeros((B, T, XD), np.float32)
    n = min(nsteps + 1, T)
    bias_drift = ((tv[:n] - tv[0])[:, None] * b4eff[None, :]).astype(np.float32)
    for c in range(NCORES):
        raw = res.results[c]["xout_d"]            # [8, nslot+16, PB]
        traj = raw.transpose(2, 1, 0)             # [PB, nslot+16, 8]
        out[c * PB:(c + 1) * PB, 0] = x[c * PB:(c + 1) * PB, 0]
        out[c * PB:(c + 1) * PB, 1:n] = traj[:, 1:n] + bias_drift[None, 1:n]
    return out


# revision 8
# speedup vs baseline: 1.0016x; 1.0016x over previous
"""Trainium2 Bass kernel for the neural-ODE Euler integration problem.

Key idea: collapse the per-step x -> L1 -> ... -> L4 -> Euler -> x loop into a
PSUM-resident recurrence on the layer-1 preactivation a1 [256, PB]:

    a1_{i+1} = a1_i + s3_i @ G + Uz . dz_i + c_G        (G = dt * W4 @ Ux)

where s3 = elu(a3)+1 is the (shifted) last hidden activation.  The x
trajectory accumulates separately in PSUM (x_{i+1} = x_i + s3_i @ (dt*W4)),
off the critical path; the constant bias drift (t_i - t_0) * b4 is added
on the host during unpack.

elu(v)+1 = relu(v) + 1 + min(exp(v)-1, 0)   (exact), fed to the next layer
as TWO matmul rhs streams so no combine op sits on the critical path:
    ScalarE:  e  = exp(v + b)            (PSUM -> SBUF fp16)
    VectorE:  r  = relu(v + b)           (PSUM -> SBUF fp16)
    GpSimd:   n' = min(e - 1, 0)         (SBUF -> SBUF fp16, off the bank)
    next layer: s@W = r@W + n'@W + colsum(W)  (colsum folded into biases)
Each PSUM tile owns a full bank to avoid collision stalls.
"""

import numpy as np
import sys

if '/opt/trn_rl_repo' not in sys.path:
    sys.path.insert(0, '/opt/trn_rl_repo')

import concourse.bass as bass
import concourse.bacc as bacc
import concourse.mybir as mybir
from concourse.tile import TileContext
from concourse import bass_utils

F32 = mybir.dt.float32
F16 = mybir.dt.float16
AF = mybir.ActivationFunctionType
OP = mybir.AluOpType

B, T, XD, ZD, HID = 1024, 1000, 8, 8, 256
NCORES = 8
PB = B // NCORES          # 128 trajectories per core
SPG = 16                  # steps per output ring group
NG = 63                   # 63*16 = 1008 slots >= 999
NSTEPS = T - 1

LAST_RESULTS = None


def _build(nsteps, ng):
    nc = bacc.Bacc("TRN2", target_bir_lowering=False, debug=False,
                   num_devices=NCORES)
    ncols = ng * PB
    nslot = ng * SPG

    d = {}
    def din(name, shape, dt):
        d[name] = nc.dram_tensor(name, shape, dt, kind="ExternalInput").ap()
    din("dzpack", [128, ncols], F16)    # packed z_eff diffs
    din("uzd",   [128, 16 * 256], F16)  # 16 slot variants of Uz rows
    din("gw",    [128, 512], F16)       # G = dt*W4@Ux, chunk (kc,h)
    din("w2p",   [128, 512], F16)
    din("w3p",   [128, 512], F16)
    din("w4d",   [128, 16], F16)        # dt*W4, chunk kc
    din("cgl",   [128, 256], F16)       # c_G hi/lo rows, per half
    din("a10hi", [128, 2 * PB], F16)    # a1_0 halves
    din("a10lo", [128, 2 * PB], F16)
    din("x0hi",  [8, PB], F16)
    din("x0lo",  [8, PB], F16)
    din("ones2", [128, PB], F16)        # rows 0-1 = 1
    din("ident", [128, 128], F16)
    din("bvec",  [128, 8], F32)         # bias columns (see host)
    xout_d = nc.dram_tensor("xout_d", [8, nslot + SPG, PB], F32,
                            kind="ExternalOutput").ap()

    with TileContext(nc) as tc:
        with tc.tile_pool(name="const", bufs=1) as cpool, \
             tc.tile_pool(name="work", bufs=36) as wpool, \
             tc.tile_pool(name="psum", bufs=1, space="PSUM") as ppool:

            sb = {}
            for name in d:
                shape = [int(s) for s in d[name].shape]
                sb[name] = cpool.tile(shape, d[name].dtype, name=name, tag=name)
                nc.sync.dma_start(out=sb[name][:], in_=d[name])

            # PSUM: one full bank per tile
            pa = {}
            for nm in ("a1h0", "a1h1", "a2h0", "a2h1", "a3h0", "a3h1", "x"):
                pa[nm] = ppool.tile([128, 512], F32, name=nm, tag=nm)

            a1 = [pa["a1h0"][:, 0:PB], pa["a1h1"][:, 0:PB]]
            a2 = [pa["a2h0"][:, 0:PB], pa["a2h1"][:, 0:PB]]
            a3 = [pa["a3h0"][:, 0:PB], pa["a3h1"][:, 0:PB]]
            px = pa["x"][0:8, 0:PB]

            bexp = {2: [sb["bvec"][:, 0:1], sb["bvec"][:, 1:2]],
                    3: [sb["bvec"][:, 4:5], sb["bvec"][:, 5:6]]}
            brel = {2: [sb["bvec"][:, 2:3], sb["bvec"][:, 3:4]],
                    3: [sb["bvec"][:, 6:7], sb["bvec"][:, 7:8]]}

            # ---- init: a1_0 and x_0 into PSUM via identity matmuls ----
            for h in range(2):
                hs = slice(h * PB, (h + 1) * PB)
                nc.tensor.matmul(a1[h], lhsT=sb["ident"][:],
                                 rhs=sb["a10hi"][:, hs], start=True, stop=False)
                nc.tensor.matmul(a1[h], lhsT=sb["ident"][:],
                                 rhs=sb["a10lo"][:, hs], start=False, stop=True)
            nc.tensor.matmul(px, lhsT=sb["ident"][0:8, 0:8],
                             rhs=sb["x0hi"][:], start=True, stop=False)
            nc.tensor.matmul(px, lhsT=sb["ident"][0:8, 0:8],
                             rhs=sb["x0lo"][:], start=False, stop=True)

            rings = {}

            def acts(layer, src, tag):
                """emit e (ACT), r = relu(v+b) (DVE), n' = min(e-1,0) (POOL);
                return (r, n) stream tiles."""
                e, r, n = [], [], []
                for h in range(2):
                    e.append(wpool.tile([128, PB], F16, name=f"e{tag}{h}",
                                        tag=f"e{layer}h{h}"))
                    r.append(wpool.tile([128, PB], F16, name=f"r{tag}{h}",
                                        tag=f"r{layer}h{h}"))
                    n.append(wpool.tile([128, PB], F16, name=f"n{tag}{h}",
                                        tag=f"n{layer}h{h}"))
                for h in range(2):
                    if layer == 1:
                        nc.scalar.activation(e[h][:], src[h], AF.Exp)
                    else:
                        nc.scalar.activation(e[h][:], src[h], AF.Exp,
                                             bias=bexp[layer][h])
                for h in range(2):
                    if layer == 1:
                        nc.vector.tensor_scalar_max(r[h][:], src[h], 0.0)
                    else:
                        nc.vector.tensor_scalar(
                            out=r[h][:], in0=src[h], scalar1=bexp[layer][h],
                            scalar2=0.0, op0=OP.add, op1=OP.max)
                for h in range(2):
                    nc.gpsimd.tensor_scalar(
                        out=n[h][:], in0=e[h][:], scalar1=-1.0,
                        scalar2=0.0, op0=OP.add, op1=OP.min)
                return r, n

            for i in range(nsteps):
                g, st = i // SPG, i % SPG
                gc = slice(g * PB, (g + 1) * PB)

                # ---- round 1: activations of a1 ----
                r1, n1 = acts(1, a1, f"1_{i}")

                # ---- layer-2 matmuls + a1 constant updates ----
                # kc0 group (gated on h0-half streams), r before n'
                for h in range(2):
                    w = sb["w2p"][:, h * 128:(h + 1) * 128]
                    nc.tensor.matmul(a2[h], lhsT=w, rhs=r1[0][:],
                                     start=True, stop=False)
                    nc.tensor.matmul(a2[h], lhsT=w, rhs=n1[0][:],
                                     start=False, stop=False)
                nc.tensor.matmul(a1[0], lhsT=sb["cgl"][:, 0:128],
                                 rhs=sb["ones2"][:], start=False, stop=False,
                                 skip_group_check=True)
                nc.tensor.matmul(
                    a1[0],
                    lhsT=sb["uzd"][:, st * 256:st * 256 + 128],
                    rhs=sb["dzpack"][:, gc], start=False, stop=False,
                    skip_group_check=True)
                # kc1 group: r's gate lands last -> emit r first per half
                for h in range(2):
                    w = sb["w2p"][:, (2 + h) * 128:(3 + h) * 128]
                    nc.tensor.matmul(a2[h], lhsT=w, rhs=r1[1][:],
                                     start=False, stop=False)
                    nc.tensor.matmul(a2[h], lhsT=w, rhs=n1[1][:],
                                     start=False, stop=True)
                nc.tensor.matmul(a1[1], lhsT=sb["cgl"][:, 128:256],
                                 rhs=sb["ones2"][:], start=False, stop=False,
                                 skip_group_check=True)
                nc.tensor.matmul(
                    a1[1],
                    lhsT=sb["uzd"][:, st * 256 + 128:st * 256 + 256],
                    rhs=sb["dzpack"][:, gc], start=False, stop=False,
                    skip_group_check=True)

                # ---- round 2 ----
                r2, n2 = acts(2, a2, f"2_{i}")
                for h in range(2):
                    w = sb["w3p"][:, h * 128:(h + 1) * 128]
                    nc.tensor.matmul(a3[h], lhsT=w, rhs=r2[0][:],
                                     start=True, stop=False)
                    nc.tensor.matmul(a3[h], lhsT=w, rhs=n2[0][:],
                                     start=False, stop=False)
                for h in range(2):
                    w = sb["w3p"][:, (2 + h) * 128:(3 + h) * 128]
                    nc.tensor.matmul(a3[h], lhsT=w, rhs=r2[1][:],
                                     start=False, stop=False)
                    nc.tensor.matmul(a3[h], lhsT=w, rhs=n2[1][:],
                                     start=False, stop=True)

                # ---- round 3 ----
                r3, n3 = acts(3, a3, f"3_{i}")

                # ---- a1 G-update + x update ----
                for h in range(2):
                    w = sb["gw"][:, h * 128:(h + 1) * 128]
                    nc.tensor.matmul(a1[h], lhsT=w, rhs=r3[0][:],
                                     start=False, stop=False,
                                     skip_group_check=True)
                    nc.tensor.matmul(a1[h], lhsT=w, rhs=n3[0][:],
                                     start=False, stop=False,
                                     skip_group_check=True)
                nc.tensor.matmul(px, lhsT=sb["w4d"][:, 0:8],
                                 rhs=r3[0][:], start=False, stop=False,
                                 skip_group_check=True)
                nc.tensor.matmul(px, lhsT=sb["w4d"][:, 0:8],
                                 rhs=n3[0][:], start=False, stop=False,
                                 skip_group_check=True)
                for h in range(2):
                    w = sb["gw"][:, (2 + h) * 128:(3 + h) * 128]
                    nc.tensor.matmul(a1[h], lhsT=w, rhs=r3[1][:],
                                     start=False, stop=False,
                                     skip_group_check=True)
                    nc.tensor.matmul(a1[h], lhsT=w, rhs=n3[1][:],
                                     start=False, stop=True,
                                     skip_group_check=True)
                nc.tensor.matmul(px, lhsT=sb["w4d"][:, 8:16],
                                 rhs=n3[1][:], start=False, stop=False,
                                 skip_group_check=True)
                nc.tensor.matmul(px, lhsT=sb["w4d"][:, 8:16],
                                 rhs=r3[1][:], start=False, stop=True,
                                 skip_group_check=True)

                # ---- x_{i+1} into output ring ----
                if g not in rings:
                    rings[g] = wpool.tile([8, SPG * PB], F32, name="xring",
                                          tag="xring", bufs=2)
                nc.vector.tensor_copy(out=rings[g][:, st * PB:(st + 1) * PB],
                                      in_=px)
                if st == SPG - 1 or i == nsteps - 1:
                    nfill = st + 1
                    nc.sync.dma_start(
                        out=xout_d[:, g * SPG + 1:g * SPG + 1 + nfill, :],
                        in_=rings[g][:, :nfill * PB])

    nc.compile()
    return nc


_BUILD_CACHE = {}


def _get_compiled(nsteps, ng):
    key = (nsteps, ng)
    if key not in _BUILD_CACHE:
        _BUILD_CACHE[key] = _build(nsteps, ng)
    return _BUILD_CACHE[key]


def _pack_stream(a, ng):
    """[PB, ng*16, 8] -> [128, ng*128]: row s*8+f, col g*128+j."""
    pb = a.shape[0]
    return (a.transpose(1, 2, 0)
             .reshape(ng, SPG, 8, pb)
             .transpose(1, 2, 0, 3)
             .reshape(128, ng * pb))


def _hilo16(v):
    hi = v.astype(np.float16)
    lo = (v.astype(np.float32) - hi.astype(np.float32)).astype(np.float16)
    return hi, lo


def make_in_maps(t, x, z, event_t, z_jump, W1, b1, W2, b2, W3, b3, W4, b4,
                 nsteps=NSTEPS, ng=NG):
    t = np.asarray(t, np.float32); x = np.asarray(x, np.float32)
    z = np.asarray(z, np.float32)
    event_t = np.asarray(event_t, np.float32)
    z_jump = np.asarray(z_jump, np.float32)
    W1 = np.asarray(W1, np.float32); b1 = np.asarray(b1, np.float32)
    W2 = np.asarray(W2, np.float32); b2 = np.asarray(b2, np.float32)
    W3 = np.asarray(W3, np.float32); b3 = np.asarray(b3, np.float32)
    W4 = np.asarray(W4, np.float32); b4 = np.asarray(b4, np.float32)

    nslot = ng * SPG
    tv = t[0, :, 0]
    dtc = np.float32((tv[nsteps] - tv[0]) / nsteps)

    W1a, W1b, W1c = W1[0:16], W1[16:32], W1[32:48]
    V = (W1a - W1b).astype(np.float64)
    U = (W1b + W1c).astype(np.float64)
    Ux, Uz = U[:8], U[8:16]

    G = dtc * (W4.astype(np.float64) @ Ux)                    # [256, 256]
    cG = dtc * (b4.astype(np.float64) @ Ux)                   # [256]
    cGhi, cGlo = _hilo16(cG)

    gw = G.reshape(2, 128, 2, 128).transpose(1, 0, 2, 3).reshape(128, 512)
    w2p = W2.reshape(2, 128, 2, 128).transpose(1, 0, 2, 3).reshape(128, 512)
    w3p = W3.reshape(2, 128, 2, 128).transpose(1, 0, 2, 3).reshape(128, 512)
    w4d = (dtc * W4).reshape(2, 128, XD).transpose(1, 0, 2).reshape(128, 16)

    cgl = np.zeros((128, 256), np.float16)
    for h in range(2):
        cgl[0, h * 128:(h + 1) * 128] = cGhi[h * 128:(h + 1) * 128]
        cgl[1, h * 128:(h + 1) * 128] = cGlo[h * 128:(h + 1) * 128]

    uz16 = Uz.astype(np.float16)
    uzd = np.zeros((128, 16, 256), np.float16)
    for r in range(16):
        uzd[8 * r:8 * r + 8, r] = uz16
    uzd = uzd.reshape(128, 16 * 256)

    # s = r + n' + 1: the ones-stream colsum cancels the elu-shift colsum,
    # so layer biases stay plain b2/b3.
    b2e = b2.astype(np.float32)
    b3e = b3.astype(np.float32)
    bvec = np.zeros((128, 8), np.float32)
    bvec[:, 0] = b2e[0:128];  bvec[:, 1] = b2e[128:256]
    bvec[:, 4] = b3e[0:128];  bvec[:, 5] = b3e[128:256]

    ident = np.eye(128, dtype=np.float16)
    ones2 = np.concatenate([np.ones((2, PB), np.float16),
                            np.zeros((126, PB), np.float16)], axis=0)

    shared = dict(gw=gw.astype(np.float16), w2p=w2p.astype(np.float16),
                  w3p=w3p.astype(np.float16), w4d=w4d.astype(np.float16),
                  cgl=cgl, uzd=uzd, bvec=bvec, ident=ident, ones2=ones2)

    # z_eff diffs per trajectory
    nz = min(nsteps, T - 1)
    zeff = np.where(tv[None, :nz, None] >= event_t[:, :, None],
                    z_jump[:, None, :], z[:, :nz])
    dz_full = np.zeros((B, nslot, ZD), np.float32)
    dz_full[:, :nz - 1] = zeff[:, 1:] - zeff[:, :-1]

    in_maps = []
    for c in range(NCORES):
        bs = slice(c * PB, (c + 1) * PB)
        x0 = x[bs, 0]
        z0 = z[bs, 0]
        all_init = np.concatenate([x0, z0], -1).astype(np.float64)
        a10 = (all_init @ (V + U) + b1).astype(np.float32)    # [PB, 256]
        a10hi, a10lo = _hilo16(a10.T)                          # [256, PB]
        a10hi2 = np.concatenate([a10hi[0:128], a10hi[128:256]], axis=1)
        a10lo2 = np.concatenate([a10lo[0:128], a10lo[128:256]], axis=1)
        x0hi, x0lo = _hilo16(x0.T)
        m = dict(shared)
        m.update(dzpack=_pack_stream(dz_full[bs].astype(np.float16), ng),
                 a10hi=a10hi2, a10lo=a10lo2, x0hi=x0hi, x0lo=x0lo)
        in_maps.append({k: np.ascontiguousarray(v) for k, v in m.items()})
    return in_maps, tv, b4.astype(np.float64)


def kernel(t, x, z, event_t, z_jump, W1, b1, W2, b2, W3, b3, W4, b4,
           nsteps=NSTEPS, ng=NG):
    global LAST_RESULTS
    x = np.asarray(x, np.float32)
    in_maps, tv, b4eff = make_in_maps(
        t, x, z, event_t, z_jump, W1, b1, W2, b2, W3, b3, W4, b4,
        nsteps=nsteps, ng=ng)

    nc = _get_compiled(nsteps, ng)
    res = bass_utils.run_bass_kernel_spmd(nc, in_maps,
                                          core_ids=list(range(NCORES)))
    LAST_RESULTS = res

    out = np.zeros((B, T, XD), np.float32)
    n = min(nsteps + 1, T)
    bias_drift = ((tv[:n] - tv[0])[:, None] * b4eff[None, :]).astype(np.float32)
    for c in range(NCORES):
        raw = res.results[c]["xout_d"]            # [8, nslot+16, PB]
        traj = raw.transpose(2, 1, 0)             # [PB, nslot+16, 8]
        out[c * PB:(c + 1) * PB, 0] = x[c * PB:(c + 1) * PB, 0]
        out[c * PB:(c + 1) * PB, 1:n] = traj[:, 1:n] + bias_drift[None, 1:n]
    return out


# revision 9
# speedup vs baseline: 1.0016x; 1.0000x over previous
"""Trainium2 Bass kernel for the neural-ODE Euler integration problem.

Key idea: collapse the per-step x -> L1 -> ... -> L4 -> Euler -> x loop into a
PSUM-resident recurrence on the layer-1 preactivation a1 [256, PB]:

    a1_{i+1} = a1_i + s3_i @ G + Uz . dz_i + c_G        (G = dt * W4 @ Ux)

where s3 = elu(a3)+1 is the (shifted) last hidden activation.  The x
trajectory accumulates separately in PSUM (x_{i+1} = x_i + s3_i @ (dt*W4)),
off the critical path; the constant bias drift (t_i - t_0) * b4 is added
on the host during unpack.

elu(v)+1 = relu(v) + 1 + min(exp(v)-1, 0)   (exact), fed to the next layer
as TWO matmul rhs streams so no combine op sits on the critical path:
    ScalarE:  e  = exp(v + b)            (PSUM -> SBUF fp16)
    VectorE:  r  = relu(v + b)           (PSUM -> SBUF fp16)
    GpSimd:   n' = min(e - 1, 0)         (SBUF -> SBUF fp16, off the bank)
    next layer: s@W = r@W + n'@W + colsum(W)  (colsum folded into biases)
Each PSUM tile owns a full bank to avoid collision stalls.
"""

import numpy as np
import sys

if '/opt/trn_rl_repo' not in sys.path:
    sys.path.insert(0, '/opt/trn_rl_repo')

import concourse.bass as bass
import concourse.bacc as bacc
import concourse.mybir as mybir
from concourse.tile import TileContext
from concourse import bass_utils

F32 = mybir.dt.float32
F16 = mybir.dt.float16
AF = mybir.ActivationFunctionType
OP = mybir.AluOpType

B, T, XD, ZD, HID = 1024, 1000, 8, 8, 256
NCORES = 8
PB = B // NCORES          # 128 trajectories per core
SPG = 16                  # steps per output ring group
NG = 63                   # 63*16 = 1008 slots >= 999
NSTEPS = T - 1

LAST_RESULTS = None


def _build(nsteps, ng):
    nc = bacc.Bacc("TRN2", target_bir_lowering=False, debug=False,
                   num_devices=NCORES)
    ncols = ng * PB
    nslot = ng * SPG

    d = {}
    def din(name, shape, dt):
        d[name] = nc.dram_tensor(name, shape, dt, kind="ExternalInput").ap()
    din("dzpack", [128, ncols], F16)    # packed z_eff diffs
    din("uzd",   [128, 16 * 256], F16)  # 16 slot variants of Uz rows
    din("gw",    [128, 512], F16)       # G = dt*W4@Ux, chunk (kc,h)
    din("w2p",   [128, 512], F16)
    din("w3p",   [128, 512], F16)
    din("w4d",   [128, 16], F16)        # dt*W4, chunk kc
    din("cgl",   [128, 256], F16)       # c_G hi/lo rows, per half
    din("a10hi", [128, 2 * PB], F16)    # a1_0 halves
    din("a10lo", [128, 2 * PB], F16)
    din("x0hi",  [8, PB], F16)
    din("x0lo",  [8, PB], F16)
    din("ones2", [128, PB], F16)        # rows 0-1 = 1
    din("ident", [128, 128], F16)
    din("bvec",  [128, 8], F32)         # bias columns (see host)
    xout_d = nc.dram_tensor("xout_d", [8, nslot + SPG, PB], F32,
                            kind="ExternalOutput").ap()

    with TileContext(nc) as tc:
        with tc.tile_pool(name="const", bufs=1) as cpool, \
             tc.tile_pool(name="work", bufs=36) as wpool, \
             tc.tile_pool(name="psum", bufs=1, space="PSUM") as ppool:

            sb = {}
            # allocate in declaration order, but DMA small/init-critical
            # tensors first: the big dzpack/uzd streams otherwise head the
            # DMA queue and stall the init matmuls behind ~3MB of traffic.
            for name in d:
                shape = [int(s) for s in d[name].shape]
                sb[name] = cpool.tile(shape, d[name].dtype, name=name, tag=name)
            _dma_order = [n for n in d if n not in ("dzpack", "uzd")] + \
                         ["uzd", "dzpack"]
            for name in _dma_order:
                nc.sync.dma_start(out=sb[name][:], in_=d[name])

            # PSUM: one full bank per tile
            pa = {}
            for nm in ("a1h0", "a1h1", "a2h0", "a2h1", "a3h0", "a3h1", "x"):
                pa[nm] = ppool.tile([128, 512], F32, name=nm, tag=nm)

            a1 = [pa["a1h0"][:, 0:PB], pa["a1h1"][:, 0:PB]]
            a2 = [pa["a2h0"][:, 0:PB], pa["a2h1"][:, 0:PB]]
            a3 = [pa["a3h0"][:, 0:PB], pa["a3h1"][:, 0:PB]]
            px = pa["x"][0:8, 0:PB]

            bexp = {2: [sb["bvec"][:, 0:1], sb["bvec"][:, 1:2]],
                    3: [sb["bvec"][:, 4:5], sb["bvec"][:, 5:6]]}
            brel = {2: [sb["bvec"][:, 2:3], sb["bvec"][:, 3:4]],
                    3: [sb["bvec"][:, 6:7], sb["bvec"][:, 7:8]]}

            # ---- init: a1_0 and x_0 into PSUM via identity matmuls ----
            for h in range(2):
                hs = slice(h * PB, (h + 1) * PB)
                nc.tensor.matmul(a1[h], lhsT=sb["ident"][:],
                                 rhs=sb["a10hi"][:, hs], start=True, stop=False)
                nc.tensor.matmul(a1[h], lhsT=sb["ident"][:],
                                 rhs=sb["a10lo"][:, hs], start=False, stop=True)
            nc.tensor.matmul(px, lhsT=sb["ident"][0:8, 0:8],
                             rhs=sb["x0hi"][:], start=True, stop=False)
            nc.tensor.matmul(px, lhsT=sb["ident"][0:8, 0:8],
                             rhs=sb["x0lo"][:], start=False, stop=True)

            rings = {}

            def acts(layer, src, tag):
                """emit e (ACT), r = relu(v+b) (DVE), n' = min(e-1,0) (POOL);
                return (r, n) stream tiles."""
                e, r, n = [], [], []
                for h in range(2):
                    e.append(wpool.tile([128, PB], F16, name=f"e{tag}{h}",
                                        tag=f"e{layer}h{h}"))
                    r.append(wpool.tile([128, PB], F16, name=f"r{tag}{h}",
                                        tag=f"r{layer}h{h}"))
                    n.append(wpool.tile([128, PB], F16, name=f"n{tag}{h}",
                                        tag=f"n{layer}h{h}"))
                for h in range(2):
                    if layer == 1:
                        nc.scalar.activation(e[h][:], src[h], AF.Exp)
                    else:
                        nc.scalar.activation(e[h][:], src[h], AF.Exp,
                                             bias=bexp[layer][h])
                for h in range(2):
                    if layer == 1:
                        nc.vector.tensor_scalar_max(r[h][:], src[h], 0.0)
                    else:
                        nc.vector.tensor_scalar(
                            out=r[h][:], in0=src[h], scalar1=bexp[layer][h],
                            scalar2=0.0, op0=OP.add, op1=OP.max)
                for h in range(2):
                    nc.gpsimd.tensor_scalar(
                        out=n[h][:], in0=e[h][:], scalar1=-1.0,
                        scalar2=0.0, op0=OP.add, op1=OP.min)
                return r, n

            for i in range(nsteps):
                g, st = i // SPG, i % SPG
                gc = slice(g * PB, (g + 1) * PB)

                # ---- round 1: activations of a1 ----
                r1, n1 = acts(1, a1, f"1_{i}")

                # ---- layer-2 matmuls + a1 constant updates ----
                # kc0 group (gated on h0-half streams), r before n'
                for h in range(2):
                    w = sb["w2p"][:, h * 128:(h + 1) * 128]
                    nc.tensor.matmul(a2[h], lhsT=w, rhs=r1[0][:],
                                     start=True, stop=False)
                    nc.tensor.matmul(a2[h], lhsT=w, rhs=n1[0][:],
                                     start=False, stop=False)
                nc.tensor.matmul(a1[0], lhsT=sb["cgl"][:, 0:128],
                                 rhs=sb["ones2"][:], start=False, stop=False,
                                 skip_group_check=True)
                nc.tensor.matmul(
                    a1[0],
                    lhsT=sb["uzd"][:, st * 256:st * 256 + 128],
                    rhs=sb["dzpack"][:, gc], start=False, stop=False,
                    skip_group_check=True)
                # kc1 group: r's gate lands last -> emit r first per half
                for h in range(2):
                    w = sb["w2p"][:, (2 + h) * 128:(3 + h) * 128]
                    nc.tensor.matmul(a2[h], lhsT=w, rhs=r1[1][:],
                                     start=False, stop=False)
                    nc.tensor.matmul(a2[h], lhsT=w, rhs=n1[1][:],
                                     start=False, stop=True)
                nc.tensor.matmul(a1[1], lhsT=sb["cgl"][:, 128:256],
                                 rhs=sb["ones2"][:], start=False, stop=False,
                                 skip_group_check=True)
                nc.tensor.matmul(
                    a1[1],
                    lhsT=sb["uzd"][:, st * 256 + 128:st * 256 + 256],
                    rhs=sb["dzpack"][:, gc], start=False, stop=False,
                    skip_group_check=True)

                # ---- round 2 ----
                r2, n2 = acts(2, a2, f"2_{i}")
                for h in range(2):
                    w = sb["w3p"][:, h * 128:(h + 1) * 128]
                    nc.tensor.matmul(a3[h], lhsT=w, rhs=r2[0][:],
                                     start=True, stop=False)
                    nc.tensor.matmul(a3[h], lhsT=w, rhs=n2[0][:],
                                     start=False, stop=False)
                for h in range(2):
                    w = sb["w3p"][:, (2 + h) * 128:(3 + h) * 128]
                    nc.tensor.matmul(a3[h], lhsT=w, rhs=r2[1][:],
                                     start=False, stop=False)
                    nc.tensor.matmul(a3[h], lhsT=w, rhs=n2[1][:],
                                     start=False, stop=True)

                # ---- round 3 ----
                r3, n3 = acts(3, a3, f"3_{i}")

                # ---- a1 G-update + x update ----
                for h in range(2):
                    w = sb["gw"][:, h * 128:(h + 1) * 128]
                    nc.tensor.matmul(a1[h], lhsT=w, rhs=r3[0][:],
                                     start=False, stop=False,
                                     skip_group_check=True)
                    nc.tensor.matmul(a1[h], lhsT=w, rhs=n3[0][:],
                                     start=False, stop=False,
                                     skip_group_check=True)
                nc.tensor.matmul(px, lhsT=sb["w4d"][:, 0:8],
                                 rhs=r3[0][:], start=False, stop=False,
                                 skip_group_check=True)
                nc.tensor.matmul(px, lhsT=sb["w4d"][:, 0:8],
                                 rhs=n3[0][:], start=False, stop=False,
                                 skip_group_check=True)
                for h in range(2):
                    w = sb["gw"][:, (2 + h) * 128:(3 + h) * 128]
                    nc.tensor.matmul(a1[h], lhsT=w, rhs=r3[1][:],
                                     start=False, stop=False,
                                     skip_group_check=True)
                    nc.tensor.matmul(a1[h], lhsT=w, rhs=n3[1][:],
                                     start=False, stop=True,
                                     skip_group_check=True)
                nc.tensor.matmul(px, lhsT=sb["w4d"][:, 8:16],
                                 rhs=n3[1][:], start=False, stop=False,
                                 skip_group_check=True)
                nc.tensor.matmul(px, lhsT=sb["w4d"][:, 8:16],
                                 rhs=r3[1][:], start=False, stop=True,
                                 skip_group_check=True)

                # ---- x_{i+1} into output ring ----
                if g not in rings:
                    rings[g] = wpool.tile([8, SPG * PB], F32, name="xring",
                                          tag="xring", bufs=2)
                nc.vector.tensor_copy(out=rings[g][:, st * PB:(st + 1) * PB],
                                      in_=px)
                if st == SPG - 1 or i == nsteps - 1:
                    nfill = st + 1
                    nc.sync.dma_start(
                        out=xout_d[:, g * SPG + 1:g * SPG + 1 + nfill, :],
                        in_=rings[g][:, :nfill * PB])

    nc.compile()
    return nc


_BUILD_CACHE = {}


def _get_compiled(nsteps, ng):
    key = (nsteps, ng)
    if key not in _BUILD_CACHE:
        _BUILD_CACHE[key] = _build(nsteps, ng)
    return _BUILD_CACHE[key]


def _pack_stream(a, ng):
    """[PB, ng*16, 8] -> [128, ng*128]: row s*8+f, col g*128+j."""
    pb = a.shape[0]
    return (a.transpose(1, 2, 0)
             .reshape(ng, SPG, 8, pb)
             .transpose(1, 2, 0, 3)
             .reshape(128, ng * pb))


def _hilo16(v):
    hi = v.astype(np.float16)
    lo = (v.astype(np.float32) - hi.astype(np.float32)).astype(np.float16)
    return hi, lo


def make_in_maps(t, x, z, event_t, z_jump, W1, b1, W2, b2, W3, b3, W4, b4,
                 nsteps=NSTEPS, ng=NG):
    t = np.asarray(t, np.float32); x = np.asarray(x, np.float32)
    z = np.asarray(z, np.float32)
    event_t = np.asarray(event_t, np.float32)
    z_jump = np.asarray(z_jump, np.float32)
    W1 = np.asarray(W1, np.float32); b1 = np.asarray(b1, np.float32)
    W2 = np.asarray(W2, np.float32); b2 = np.asarray(b2, np.float32)
    W3 = np.asarray(W3, np.float32); b3 = np.asarray(b3, np.float32)
    W4 = np.asarray(W4, np.float32); b4 = np.asarray(b4, np.float32)

    nslot = ng * SPG
    tv = t[0, :, 0]
    dtc = np.float32((tv[nsteps] - tv[0]) / nsteps)

    W1a, W1b, W1c = W1[0:16], W1[16:32], W1[32:48]
    V = (W1a - W1b).astype(np.float64)
    U = (W1b + W1c).astype(np.float64)
    Ux, Uz = U[:8], U[8:16]

    G = dtc * (W4.astype(np.float64) @ Ux)                    # [256, 256]
    cG = dtc * (b4.astype(np.float64) @ Ux)                   # [256]
    cGhi, cGlo = _hilo16(cG)

    gw = G.reshape(2, 128, 2, 128).transpose(1, 0, 2, 3).reshape(128, 512)
    w2p = W2.reshape(2, 128, 2, 128).transpose(1, 0, 2, 3).reshape(128, 512)
    w3p = W3.reshape(2, 128, 2, 128).transpose(1, 0, 2, 3).reshape(128, 512)
    w4d = (dtc * W4).reshape(2, 128, XD).transpose(1, 0, 2).reshape(128, 16)

    cgl = np.zeros((128, 256), np.float16)
    for h in range(2):
        cgl[0, h * 128:(h + 1) * 128] = cGhi[h * 128:(h + 1) * 128]
        cgl[1, h * 128:(h + 1) * 128] = cGlo[h * 128:(h + 1) * 128]

    uz16 = Uz.astype(np.float16)
    uzd = np.zeros((128, 16, 256), np.float16)
    for r in range(16):
        uzd[8 * r:8 * r + 8, r] = uz16
    uzd = uzd.reshape(128, 16 * 256)

    # s = r + n' + 1: the ones-stream colsum cancels the elu-shift colsum,
    # so layer biases stay plain b2/b3.
    b2e = b2.astype(np.float32)
    b3e = b3.astype(np.float32)
    bvec = np.zeros((128, 8), np.float32)
    bvec[:, 0] = b2e[0:128];  bvec[:, 1] = b2e[128:256]
    bvec[:, 4] = b3e[0:128];  bvec[:, 5] = b3e[128:256]

    ident = np.eye(128, dtype=np.float16)
    ones2 = np.concatenate([np.ones((2, PB), np.float16),
                            np.zeros((126, PB), np.float16)], axis=0)

    shared = dict(gw=gw.astype(np.float16), w2p=w2p.astype(np.float16),
                  w3p=w3p.astype(np.float16), w4d=w4d.astype(np.float16),
                  cgl=cgl, uzd=uzd, bvec=bvec, ident=ident, ones2=ones2)

    # z_eff diffs per trajectory
    nz = min(nsteps, T - 1)
    zeff = np.where(tv[None, :nz, None] >= event_t[:, :, None],
                    z_jump[:, None, :], z[:, :nz])
    dz_full = np.zeros((B, nslot, ZD), np.float32)
    dz_full[:, :nz - 1] = zeff[:, 1:] - zeff[:, :-1]

    in_maps = []
    for c in range(NCORES):
        bs = slice(c * PB, (c + 1) * PB)
        x0 = x[bs, 0]
        z0 = z[bs, 0]
        all_init = np.concatenate([x0, z0], -1).astype(np.float64)
        a10 = (all_init @ (V + U) + b1).astype(np.float32)    # [PB, 256]
        a10hi, a10lo = _hilo16(a10.T)                          # [256, PB]
        a10hi2 = np.concatenate([a10hi[0:128], a10hi[128:256]], axis=1)
        a10lo2 = np.concatenate([a10lo[0:128], a10lo[128:256]], axis=1)
        x0hi, x0lo = _hilo16(x0.T)
        m = dict(shared)
        m.update(dzpack=_pack_stream(dz_full[bs].astype(np.float16), ng),
                 a10hi=a10hi2, a10lo=a10lo2, x0hi=x0hi, x0lo=x0lo)
        in_maps.append({k: np.ascontiguousarray(v) for k, v in m.items()})
    return in_maps, tv, b4.astype(np.float64)


def kernel(t, x, z, event_t, z_jump, W1, b1, W2, b2, W3, b3, W4, b4,
           nsteps=NSTEPS, ng=NG):
    global LAST_RESULTS
    x = np.asarray(x, np.float32)
    in_maps, tv, b4eff = make_in_maps(
        t, x, z, event_t, z_jump, W1, b1, W2, b2, W3, b3, W4, b4,
        nsteps=nsteps, ng=ng)

    nc = _get_compiled(nsteps, ng)
    res = bass_utils.run_bass_kernel_spmd(nc, in_maps,
                                          core_ids=list(range(NCORES)))
    LAST_RESULTS = res

    out = np.zeros((B, T, XD), np.float32)
    n = min(nsteps + 1, T)
    bias_drift = ((tv[:n] - tv[0])[:, None] * b4eff[None, :]).astype(np.float32)
    for c in range(NCORES):
        raw = res.results[c]["xout_d"]            # [8, nslot+16, PB]
        traj = raw.transpose(2, 1, 0)             # [PB, nslot+16, 8]
        out[c * PB:(c + 1) * PB, 0] = x[c * PB:(c + 1) * PB, 0]
        out[c * PB:(c + 1) * PB, 1:n] = traj[:, 1:n] + bias_drift[None, 1:n]
    return out
